# revision 13
# baseline (speedup 1.0000x reference)
"""Trainium2 Bass kernel for ComputeVecSimilarityLoss.

Reference semantics (B batches, N points, D=2):
    sm      = where(cos < th, 0, cos)                      [B,N,N]
    v[i,j]  = (gt[i] - gt[j]) * sm[i,j]  -> [B, M=N*N, D]
    dot     = v @ v^T per batch                            [B,M,M]
    idx_num = count(dot != 0)
    vabs    = sqrt(sum(v*v + 1e-9, axis=D))
    result  = sum(|dot| / (vabs_m*vabs_n)) / idx_num

Restructuring (mathematically exact, fp-equal to ~1e-6):
  * u = v / vabs  (host, O(B*M)): |dot|/(vabs_m*vabs_n) == |u_m . u_n|.
  * zero vectors are compacted away on the host; idx_num = sum_b nnz_b^2.
  * batch b -> NeuronCore b (pure data parallel, B == 8 cores).
    Each core computes S_b = sum |u_m . u_n| over its compacted M_b x M_b
    block; host does the final scalar division.

Device kernel per core (v2):
  * u is replicated by the host into 4 partition groups {0,32,64,96} so
    the PE array runs as 16 concurrent 32x32 tiles (tile_position row =
    data's partition group, col = output 32-row strip).  A second copy
    scaled by 0.5 provides the weights for diagonal blocks, so every
    PSUM value can be summed by any consumer engine without per-group
    scales (host multiplies the grand total by 2 at the end).
  * The upper triangle of the M x M |dot| matrix is covered by
    128-row x (<=512)-col blocks; each block is 4 strip-matmuls.  Blocks
    are packed bank-aligned into [128, 2048] PSUM supertiles.
  * ScalarE (Abs activation + accum_out) and VectorE (tensor_reduce with
    apply_absolute_value) split the PSUM supertiles; a final VectorE
    reduce collapses the per-consumer partials to a [128, 1] output.
"""

import os

import numpy as np

EPS = np.float32(1e-9)
N_CORES = 8
BANK = 512           # fp32 elements per PSUM bank per partition
SUPER = 2048         # PSUM supertile columns (4 banks)

LAST_RESULTS = None

_PROGRAM_CACHE = {}


def _plan(P, COLS):
    """Plan the triangular coverage.

    Returns (jobs per supertile, consumer specs).  A job is
    (row_tile, col0, width, psum_off, is_diag); every job's PSUM span
    [psum_off, psum_off+width) stays inside one 512-wide bank slot.
    A consumer spec is (tile_idx, off, k, w, engine) meaning a 3D AP
    [128, k, w] at stride BANK (k==1 -> plain 2D span) reduced by
    'engine' ('act' or 'dve').
    """
    n_tiles = P // 128

    # per row-tile: diagonal 128x128 block + right-aligned above-diagonal
    # chunks (<=512 wide, first chunk absorbs the remainder), trimmed to
    # COLS on the right edge.
    jobs = []  # (t, col0, w, is_diag)
    for t in range(n_tiles):
        d_w = min(128, max(0, COLS - 128 * t))
        if d_w > 0:
            jobs.append((t, 128 * t, d_w, True))
        s = 128 * (t + 1)
        W = COLS - s
        if W <= 0:
            continue
        w0 = W % BANK
        c = s
        if w0:
            jobs.append((t, c, w0, False))
            c += w0
        while c < COLS:
            jobs.append((t, c, BANK, False))
            c += BANK

    # row-group assignment: greedy balance of per-row-tile streamed cols
    loads = [0.0] * 2
    row_group = [0] * n_tiles
    per_t = [sum(w for (t, _, w, _) in jobs if t == tt) for tt in range(n_tiles)]
    for tt in sorted(range(n_tiles), key=lambda x: -per_t[x]):
        g = min(range(len(loads)), key=lambda x: loads[x])
        row_group[tt] = g
        loads[g] += per_t[tt]

    # PSUM packing: equal-width classes -> 512-strided slots (3D consumer),
    # small widths (<= 256) pack contiguously inside shared banks.
    by_w = {}
    for j in jobs:
        by_w.setdefault(j[2], []).append(j)

    supertiles = []  # list of lists of (t, col0, w, off, is_diag)
    consumers = []   # (tile_idx, off, k, w, engine placeholder)
    cur = []         # jobs in current supertile
    cur_off = 0

    def flush():
        nonlocal cur, cur_off
        if cur:
            supertiles.append(cur)
            cur = []
            cur_off = 0

    # big widths first (one bank slot each)
    for w in sorted(by_w, reverse=True):
        if w <= 256:
            continue
        pend = by_w.pop(w)
        while pend:
            space = (SUPER - cur_off) // BANK
            take = pend[:space] if space else []
            if not take:
                flush()
                continue
            k = len(take)
            for i, (t, c0, ww, dg) in enumerate(take):
                cur.append((t, c0, ww, cur_off + i * BANK, dg))
            consumers.append([len(supertiles), cur_off, k, w, None])
            cur_off += k * BANK
            pend = pend[len(take):]
            if cur_off >= SUPER:
                flush()
    # small widths: contiguous packing, never crossing a bank boundary
    small = [j for w in sorted(by_w, reverse=True) for j in by_w[w]]
    run_start = None
    run_k = 0
    run_w = 0
    for t, c0, w, dg in small:
        bank_rem = BANK - (cur_off % BANK)
        if w > bank_rem:
            # skip to the next bank; the gap ends the contiguous run
            if run_k:
                consumers.append([len(supertiles), run_start, 1, run_w, None])
                run_k = 0
            cur_off += bank_rem
        if cur_off + w > SUPER:
            if run_k:
                consumers.append([len(supertiles), run_start, 1, run_w, None])
                run_k = 0
            flush()
        if run_k == 0:
            run_start, run_w = cur_off, 0
        cur.append((t, c0, w, cur_off, dg))
        run_w = cur_off + w - run_start
        run_k += 1
        cur_off += w
    if run_k:
        consumers.append([len(supertiles), run_start, 1, run_w, None])
    flush()

    # engine assignment: brute-force min-makespan over the consumer list
    def act_ns(e):
        return 283.0 + (172.0 + e) / 1.2

    def dve_ns(e):
        return 1.03 * (120.0 + e) / 0.96

    n = len(consumers)
    best = (float("inf"), 0)
    for mask in range(1 << n):
        a = sum(act_ns(c[2] * c[3]) for i, c in enumerate(consumers) if mask >> i & 1)
        d = sum(dve_ns(c[2] * c[3]) for i, c in enumerate(consumers) if not mask >> i & 1)
        m = max(a, d)
        if m < best[0]:
            best = (m, mask)
    for i, c in enumerate(consumers):
        c[4] = "act" if best[1] >> i & 1 else "dve"

    # PSUM is bank-granular; round up so strided consumer views stay legal
    tile_widths = [
        -(-max(off + w for (_, _, w, off, _) in st) // BANK) * BANK
        for st in supertiles
    ]
    return supertiles, consumers, row_group, tile_widths


def _build_program(P, COLS):
    key = (P, COLS)
    if key in _PROGRAM_CACHE:
        return _PROGRAM_CACHE[key]

    import concourse.bass as bass
    import concourse.mybir as mybir
    import concourse.tile as tile
    from concourse import bacc

    f32 = mybir.dt.float32
    f16 = mybir.dt.float16
    supertiles, consumers, row_group, tile_widths = _plan(P, COLS)
    ncol = sum(c[2] if c[4] == "dve" else 1 for c in consumers)

    nc = bacc.Bacc(
        "TRN2",
        target_bir_lowering=False,
        debug=False,
        enable_asserts=False,
        num_devices=N_CORES,
    )
    # [8, 2P]: rows 2r+j hold [u_j | 0.5*u_j] for replica r
    u_dram = nc.dram_tensor("u", [4, 2 * P], f16, kind="ExternalInput")
    out_dram = nc.dram_tensor("out", [128, 1], f32, kind="ExternalOutput")

    with tile.TileContext(nc) as tc:
        with (
            tc.tile_pool(name="const", bufs=1) as const_pool,
            tc.tile_pool(name="psum", bufs=2, space="PSUM") as psum_pool,
        ):
            u = const_pool.tile([128, 2 * P], f16)
            # one plain 2-partition DMA per replica, on three parallel
            # queues (partition-strided single-DMA replication is not
            # supported: the sim flags it and hardware NRT-errors)
            for r, eng in enumerate((nc.sync, nc.gpsimd)):
                eng.dma_start(
                    u[32 * r : 32 * r + 2, :], u_dram.ap()[2 * r : 2 * r + 2, :]
                )
            partials = const_pool.tile([128, ncol], f32)
            final = const_pool.tile([128, 1], f32)

            col = 0
            cons_by_tile = {}
            for ci, (ti, off, k, w, eng) in enumerate(consumers):
                cons_by_tile.setdefault(ti, []).append((ci, off, k, w, eng, col))
                # DVE tensor_reduce emits one column per chunk; ACT
                # accum_out collapses the whole group into one column
                col += k if eng == "dve" else 1

            for ti, st in enumerate(supertiles):
                ps = psum_pool.tile([128, tile_widths[ti]], f32, tag="ps")
                for t, c0, w, off, dg in st:
                    r = (ti * 4 + off // BANK) % 2
                    wbase = P if dg else 0  # diag weights read the 0.5*u copy
                    for c in range(4):
                        nc.tensor.matmul(
                            ps[32 * c : 32 * c + 32, off : off + w],
                            u[
                                32 * r : 32 * r + 2,
                                wbase + 128 * t + 32 * c : wbase + 128 * t + 32 * c + 32,
                            ],
                            u[32 * r : 32 * r + 2, c0 : c0 + w],
                            tile_position=(32 * r, 32 * c),
                        )
                for ci, off, k, w, eng, col0 in cons_by_tile.get(ti, []):
                    if k == 1:
                        src = ps[:, off : off + w]
                    else:
                        src = ps[:].rearrange("p (b n) -> p b n", n=BANK)[
                            :, off // BANK : off // BANK + k, 0:w
                        ]
                    if eng == "act":
                        nc.scalar.activation(
                            src,
                            src,
                            mybir.ActivationFunctionType.Abs,
                            accum_out=partials[:, col0 : col0 + 1],
                        )
                    else:
                        nc.vector.tensor_reduce(
                            partials[:, col0 : col0 + k],
                            src,
                            axis=mybir.AxisListType.X,
                            op=mybir.AluOpType.add,
                            apply_absolute_value=True,
                        )

            nc.vector.reduce_sum(final[:], partials[:], axis=mybir.AxisListType.X)
            nc.sync.dma_start(out_dram.ap(), final[:])

    nc.compile()
    _PROGRAM_CACHE[key] = nc
    return nc


def _preprocess(gt_points, cos_similarity, threshold):
    """Host O(B*N^2) prep: u vectors, compaction, replication, padding."""
    gt = np.asarray(gt_points, dtype=np.float32)
    cos = np.asarray(cos_similarity, dtype=np.float32)
    th = np.asarray(threshold, dtype=np.float32).reshape(-1)[0]
    B, N, D = gt.shape
    M = N * N

    sm = np.where(cos < th, np.float32(0), cos)
    v = ((gt[:, :, None, :] - gt[:, None, :, :]) * sm[..., None]).reshape(B, M, D)
    v = v.astype(np.float32)
    r2 = (v[..., 0] * v[..., 0] + EPS) + (v[..., 1] * v[..., 1] + EPS)
    vabs = np.sqrt(r2, dtype=np.float32)
    u = (v / vabs[..., None]).astype(np.float32)
    nz = np.any(v != 0, axis=-1)  # [B, M]
    nnz = nz.sum(axis=1).astype(np.int64)

    COLS = int(max(2, int(nnz.max())))
    P = int(-(-COLS // 128) * 128)

    in_maps = []
    for b in range(B):
        ub = u[b][nz[b]].T.astype(np.float16)  # [2, nnz_b]
        urep = np.zeros((4, 2 * P), dtype=np.float16)
        for r in range(2):
            urep[2 * r : 2 * r + 2, : ub.shape[1]] = ub
            urep[2 * r : 2 * r + 2, P : P + ub.shape[1]] = (
                ub.astype(np.float32) * 0.5
            ).astype(np.float16)
        in_maps.append({"u": urep})
    return in_maps, nnz, P, COLS


def _ensure_ntff_hook():
    """Shim antenv.axon_hooks if the image lacks it (profiling only)."""
    try:
        from antenv.axon_hooks import get_axon_ntff_profile_hook  # noqa: F401

        return
    except ImportError:
        pass

    import contextlib
    import ctypes
    import sys
    import types

    import antenv

    mod = types.ModuleType("antenv.axon_hooks")
    _state = {"hook": None}

    def set_axon_ntff_profile_hook(h):
        _state["hook"] = h

    def get_axon_ntff_profile_hook():
        return _state["hook"]

    mod.set_axon_ntff_profile_hook = set_axon_ntff_profile_hook
    mod.get_axon_ntff_profile_hook = get_axon_ntff_profile_hook
    sys.modules["antenv.axon_hooks"] = mod
    antenv.axon_hooks = mod

    so_path = "/opt/axon/libaxon_pjrt.so"
    if not os.path.exists(so_path):
        return
    lib = ctypes.CDLL(so_path)
    if not hasattr(lib, "axon_start_nrt_profile"):
        return
    lib.axon_start_nrt_profile.argtypes = [
        ctypes.POINTER(ctypes.c_int64),
        ctypes.c_size_t,
    ]
    lib.axon_start_nrt_profile.restype = ctypes.c_int64
    lib.axon_stop_nrt_profile.argtypes = [ctypes.c_char_p]
    lib.axon_stop_nrt_profile.restype = ctypes.c_int64

    @contextlib.contextmanager
    def _hook(output_dir, device_ids):
        import jax

        jax.devices()
        if device_ids:
            ids = (ctypes.c_int64 * len(device_ids))(*device_ids)
            rc = lib.axon_start_nrt_profile(ids, len(device_ids))
        else:
            rc = lib.axon_start_nrt_profile(None, 0)
        if rc != 0:
            raise RuntimeError(f"axon_start_nrt_profile rc={rc}")
        try:
            yield
        finally:
            n = lib.axon_stop_nrt_profile(str(output_dir).encode())
            if n < 0:
                raise RuntimeError(f"axon_stop_nrt_profile rc={n}")
            print(f"profile: {n} file(s) written to {output_dir}")

    set_axon_ntff_profile_hook(_hook)


def kernel(gt_points, cos_similarity, threshold):
    global LAST_RESULTS
    in_maps, nnz, P, COLS = _preprocess(gt_points, cos_similarity, threshold)
    B = len(in_maps)

    total_count = int((nnz.astype(np.int64) ** 2).sum())
    if total_count == 0:
        with np.errstate(invalid="ignore", divide="ignore"):
            return (np.float32(0) / np.float32(0)).astype(np.float32)

    from concourse.bass_utils import run_bass_kernel_spmd

    nc = _build_program(P, COLS)
    assert B <= N_CORES, "one batch per core"
    trace = os.environ.get("KERNEL_TRACE", "") not in ("", "0")
    if trace:
        _ensure_ntff_hook()
    res = run_bass_kernel_spmd(
        nc,
        in_maps,
        core_ids=list(range(B)),
        trace=trace,
    )
    LAST_RESULTS = res

    total = 0.0
    for b in range(B):
        out = res.results[b]["out"]
        # partials hold (upper + 0.5*diag); x2 recovers the full sum
        total += 2.0 * float(np.sum(out, dtype=np.float64))

    return np.asarray(
        np.float32(total) / np.float32(total_count), dtype=np.float32
    )


# revision 14
# speedup vs baseline: 1.1007x; 1.1007x over previous
"""Trainium2 Bass kernel for ComputeVecSimilarityLoss.

Reference semantics (B batches, N points, D=2):
    sm      = where(cos < th, 0, cos)                      [B,N,N]
    v[i,j]  = (gt[i] - gt[j]) * sm[i,j]  -> [B, M=N*N, D]
    dot     = v @ v^T per batch                            [B,M,M]
    idx_num = count(dot != 0)
    vabs    = sqrt(sum(v*v + 1e-9, axis=D))
    result  = sum(|dot| / (vabs_m*vabs_n)) / idx_num

Restructuring (mathematically exact, fp-equal to ~1e-6):
  * u = v / vabs  (host, O(B*M)): |dot|/(vabs_m*vabs_n) == |u_m . u_n|.
  * zero vectors are compacted away on the host; idx_num = sum_b nnz_b^2.
  * batch b -> NeuronCore b (pure data parallel, B == 8 cores).
    Each core computes S_b = sum |u_m . u_n| over its compacted M_b x M_b
    block; host does the final scalar division.

Device kernel per core (v2):
  * u is replicated by the host into 4 partition groups {0,32,64,96} so
    the PE array runs as 16 concurrent 32x32 tiles (tile_position row =
    data's partition group, col = output 32-row strip).  A second copy
    scaled by 0.5 provides the weights for diagonal blocks, so every
    PSUM value can be summed by any consumer engine without per-group
    scales (host multiplies the grand total by 2 at the end).
  * The upper triangle of the M x M |dot| matrix is covered by
    128-row x (<=512)-col blocks; each block is 4 strip-matmuls.  Blocks
    are packed bank-aligned into [128, 2048] PSUM supertiles.
  * ScalarE (Abs activation + accum_out) and VectorE (tensor_reduce with
    apply_absolute_value) split the PSUM supertiles; a final VectorE
    reduce collapses the per-consumer partials to a [128, 1] output.
"""

import os

import numpy as np

EPS = np.float32(1e-9)
N_CORES = 8
BANK = 512           # fp32 elements per PSUM bank per partition
SUPER = 1024         # PSUM supertile columns (2 banks)

LAST_RESULTS = None

_PROGRAM_CACHE = {}


def _plan(P, COLS):
    """Plan the triangular coverage.

    Returns (jobs per supertile, consumer specs).  A job is
    (row_tile, col0, width, psum_off, is_diag); every job's PSUM span
    [psum_off, psum_off+width) stays inside one 512-wide bank slot.
    A consumer spec is (tile_idx, off, k, w, engine) meaning a 3D AP
    [128, k, w] at stride BANK (k==1 -> plain 2D span) reduced by
    'engine' ('act' or 'dve').
    """
    n_tiles = P // 128

    # per row-tile: diagonal 128x128 block + right-aligned above-diagonal
    # chunks (<=512 wide, first chunk absorbs the remainder), trimmed to
    # COLS on the right edge.
    jobs = []  # (t, col0, w, is_diag)
    for t in range(n_tiles):
        d_w = min(128, max(0, COLS - 128 * t))
        if d_w > 0:
            jobs.append((t, 128 * t, d_w, True))
        s = 128 * (t + 1)
        W = COLS - s
        if W <= 0:
            continue
        w0 = W % BANK
        c = s
        if w0:
            jobs.append((t, c, w0, False))
            c += w0
        while c < COLS:
            jobs.append((t, c, BANK, False))
            c += BANK

    # row-group assignment: greedy balance of per-row-tile streamed cols
    loads = [0.0] * 2
    row_group = [0] * n_tiles
    per_t = [sum(w for (t, _, w, _) in jobs if t == tt) for tt in range(n_tiles)]
    for tt in sorted(range(n_tiles), key=lambda x: -per_t[x]):
        g = min(range(len(loads)), key=lambda x: loads[x])
        row_group[tt] = g
        loads[g] += per_t[tt]

    # PSUM packing: equal-width classes -> 512-strided slots (3D consumer),
    # small widths (<= 256) pack contiguously inside shared banks.
    by_w = {}
    for j in jobs:
        by_w.setdefault(j[2], []).append(j)

    supertiles = []  # list of lists of (t, col0, w, off, is_diag)
    consumers = []   # (tile_idx, off, k, w, engine placeholder)
    cur = []         # jobs in current supertile
    cur_off = 0

    def flush():
        nonlocal cur, cur_off
        if cur:
            supertiles.append(cur)
            cur = []
            cur_off = 0

    # big widths first (one bank slot each)
    for w in sorted(by_w, reverse=True):
        if w <= 256:
            continue
        pend = by_w.pop(w)
        while pend:
            space = (SUPER - cur_off) // BANK
            take = pend[:space] if space else []
            if not take:
                flush()
                continue
            k = len(take)
            for i, (t, c0, ww, dg) in enumerate(take):
                cur.append((t, c0, ww, cur_off + i * BANK, dg))
            consumers.append([len(supertiles), cur_off, k, w, None])
            cur_off += k * BANK
            pend = pend[len(take):]
            if cur_off >= SUPER:
                flush()
    # small widths: contiguous packing, never crossing a bank boundary
    small = [j for w in sorted(by_w, reverse=True) for j in by_w[w]]
    run_start = None
    run_k = 0
    run_w = 0
    for t, c0, w, dg in small:
        bank_rem = BANK - (cur_off % BANK)
        if w > bank_rem:
            # skip to the next bank; the gap ends the contiguous run
            if run_k:
                consumers.append([len(supertiles), run_start, 1, run_w, None])
                run_k = 0
            cur_off += bank_rem
        if cur_off + w > SUPER:
            if run_k:
                consumers.append([len(supertiles), run_start, 1, run_w, None])
                run_k = 0
            flush()
        if run_k == 0:
            run_start, run_w = cur_off, 0
        cur.append((t, c0, w, cur_off, dg))
        run_w = cur_off + w - run_start
        run_k += 1
        cur_off += w
    if run_k:
        consumers.append([len(supertiles), run_start, 1, run_w, None])
    flush()

    # engine assignment: brute-force min-makespan over the consumer list
    def act_ns(e):
        return 283.0 + (172.0 + e) / 1.2

    def dve_ns(e):
        return 1.03 * (120.0 + e) / 0.96

    n = len(consumers)
    best = (float("inf"), 0)
    for mask in range(1 << n):
        a = sum(act_ns(c[2] * c[3]) for i, c in enumerate(consumers) if mask >> i & 1)
        d = sum(dve_ns(c[2] * c[3]) for i, c in enumerate(consumers) if not mask >> i & 1)
        m = max(a, d)
        if m < best[0]:
            best = (m, mask)
    for i, c in enumerate(consumers):
        c[4] = "act" if best[1] >> i & 1 else "dve"

    # PSUM is bank-granular; round up so strided consumer views stay legal
    tile_widths = [
        -(-max(off + w for (_, _, w, off, _) in st) // BANK) * BANK
        for st in supertiles
    ]
    return supertiles, consumers, row_group, tile_widths


def _build_program(P, COLS):
    key = (P, COLS)
    if key in _PROGRAM_CACHE:
        return _PROGRAM_CACHE[key]

    import concourse.bass as bass
    import concourse.mybir as mybir
    import concourse.tile as tile
    from concourse import bacc

    f32 = mybir.dt.float32
    f16 = mybir.dt.float16
    supertiles, consumers, row_group, tile_widths = _plan(P, COLS)
    ncol = sum(c[2] if c[4] == "dve" else 1 for c in consumers)

    nc = bacc.Bacc(
        "TRN2",
        target_bir_lowering=False,
        debug=False,
        enable_asserts=False,
        num_devices=N_CORES,
    )
    # [8, 2P]: rows 2r+j hold [u_j | 0.5*u_j] for replica r
    u_dram = nc.dram_tensor("u", [4, 2 * P], f16, kind="ExternalInput")
    out_dram = nc.dram_tensor("out", [128, 1], f32, kind="ExternalOutput")

    with tile.TileContext(nc) as tc:
        with (
            tc.tile_pool(name="const", bufs=1) as const_pool,
            tc.tile_pool(name="psum", bufs=4, space="PSUM") as psum_pool,
        ):
            u = const_pool.tile([128, 2 * P], f16)
            # one plain 2-partition DMA per replica, on three parallel
            # queues (partition-strided single-DMA replication is not
            # supported: the sim flags it and hardware NRT-errors)
            for r, eng in enumerate((nc.sync, nc.gpsimd)):
                eng.dma_start(
                    u[32 * r : 32 * r + 2, :], u_dram.ap()[2 * r : 2 * r + 2, :]
                )
            partials = const_pool.tile([128, ncol], f32)
            final = const_pool.tile([128, 1], f32)

            col = 0
            cons_by_tile = {}
            for ci, (ti, off, k, w, eng) in enumerate(consumers):
                cons_by_tile.setdefault(ti, []).append((ci, off, k, w, eng, col))
                # DVE tensor_reduce emits one column per chunk; ACT
                # accum_out collapses the whole group into one column
                col += k if eng == "dve" else 1

            for ti, st in enumerate(supertiles):
                ps = psum_pool.tile([128, tile_widths[ti]], f32, tag="ps")
                for t, c0, w, off, dg in st:
                    r = (ti * 2 + off // BANK) % 2
                    wbase = P if dg else 0  # diag weights read the 0.5*u copy
                    for c in range(4):
                        nc.tensor.matmul(
                            ps[32 * c : 32 * c + 32, off : off + w],
                            u[
                                32 * r : 32 * r + 2,
                                wbase + 128 * t + 32 * c : wbase + 128 * t + 32 * c + 32,
                            ],
                            u[32 * r : 32 * r + 2, c0 : c0 + w],
                            tile_position=(32 * r, 32 * c),
                        )
                for ci, off, k, w, eng, col0 in cons_by_tile.get(ti, []):
                    if k == 1:
                        src = ps[:, off : off + w]
                    else:
                        src = ps[:].rearrange("p (b n) -> p b n", n=BANK)[
                            :, off // BANK : off // BANK + k, 0:w
                        ]
                    if eng == "act":
                        nc.scalar.activation(
                            src,
                            src,
                            mybir.ActivationFunctionType.Abs,
                            accum_out=partials[:, col0 : col0 + 1],
                        )
                    else:
                        nc.vector.tensor_reduce(
                            partials[:, col0 : col0 + k],
                            src,
                            axis=mybir.AxisListType.X,
                            op=mybir.AluOpType.add,
                            apply_absolute_value=True,
                        )

            nc.vector.reduce_sum(final[:], partials[:], axis=mybir.AxisListType.X)
            nc.sync.dma_start(out_dram.ap(), final[:])

    nc.compile()
    _PROGRAM_CACHE[key] = nc
    return nc


def _preprocess(gt_points, cos_similarity, threshold):
    """Host O(B*N^2) prep: u vectors, compaction, replication, padding."""
    gt = np.asarray(gt_points, dtype=np.float32)
    cos = np.asarray(cos_similarity, dtype=np.float32)
    th = np.asarray(threshold, dtype=np.float32).reshape(-1)[0]
    B, N, D = gt.shape
    M = N * N

    sm = np.where(cos < th, np.float32(0), cos)
    v = ((gt[:, :, None, :] - gt[:, None, :, :]) * sm[..., None]).reshape(B, M, D)
    v = v.astype(np.float32)
    r2 = (v[..., 0] * v[..., 0] + EPS) + (v[..., 1] * v[..., 1] + EPS)
    vabs = np.sqrt(r2, dtype=np.float32)
    u = (v / vabs[..., None]).astype(np.float32)
    nz = np.any(v != 0, axis=-1)  # [B, M]
    nnz = nz.sum(axis=1).astype(np.int64)

    COLS = int(max(2, int(nnz.max())))
    P = int(-(-COLS // 128) * 128)

    in_maps = []
    for b in range(B):
        ub = u[b][nz[b]].T.astype(np.float16)  # [2, nnz_b]
        urep = np.zeros((4, 2 * P), dtype=np.float16)
        for r in range(2):
            urep[2 * r : 2 * r + 2, : ub.shape[1]] = ub
            urep[2 * r : 2 * r + 2, P : P + ub.shape[1]] = (
                ub.astype(np.float32) * 0.5
            ).astype(np.float16)
        in_maps.append({"u": urep})
    return in_maps, nnz, P, COLS


def _ensure_ntff_hook():
    """Shim antenv.axon_hooks if the image lacks it (profiling only)."""
    try:
        from antenv.axon_hooks import get_axon_ntff_profile_hook  # noqa: F401

        return
    except ImportError:
        pass

    import contextlib
    import ctypes
    import sys
    import types

    import antenv

    mod = types.ModuleType("antenv.axon_hooks")
    _state = {"hook": None}

    def set_axon_ntff_profile_hook(h):
        _state["hook"] = h

    def get_axon_ntff_profile_hook():
        return _state["hook"]

    mod.set_axon_ntff_profile_hook = set_axon_ntff_profile_hook
    mod.get_axon_ntff_profile_hook = get_axon_ntff_profile_hook
    sys.modules["antenv.axon_hooks"] = mod
    antenv.axon_hooks = mod

    so_path = "/opt/axon/libaxon_pjrt.so"
    if not os.path.exists(so_path):
        return
    lib = ctypes.CDLL(so_path)
    if not hasattr(lib, "axon_start_nrt_profile"):
        return
    lib.axon_start_nrt_profile.argtypes = [
        ctypes.POINTER(ctypes.c_int64),
        ctypes.c_size_t,
    ]
    lib.axon_start_nrt_profile.restype = ctypes.c_int64
    lib.axon_stop_nrt_profile.argtypes = [ctypes.c_char_p]
    lib.axon_stop_nrt_profile.restype = ctypes.c_int64

    @contextlib.contextmanager
    def _hook(output_dir, device_ids):
        import jax

        jax.devices()
        if device_ids:
            ids = (ctypes.c_int64 * len(device_ids))(*device_ids)
            rc = lib.axon_start_nrt_profile(ids, len(device_ids))
        else:
            rc = lib.axon_start_nrt_profile(None, 0)
        if rc != 0:
            raise RuntimeError(f"axon_start_nrt_profile rc={rc}")
        try:
            yield
        finally:
            n = lib.axon_stop_nrt_profile(str(output_dir).encode())
            if n < 0:
                raise RuntimeError(f"axon_stop_nrt_profile rc={n}")
            print(f"profile: {n} file(s) written to {output_dir}")

    set_axon_ntff_profile_hook(_hook)


def kernel(gt_points, cos_similarity, threshold):
    global LAST_RESULTS
    in_maps, nnz, P, COLS = _preprocess(gt_points, cos_similarity, threshold)
    B = len(in_maps)

    total_count = int((nnz.astype(np.int64) ** 2).sum())
    if total_count == 0:
        with np.errstate(invalid="ignore", divide="ignore"):
            return (np.float32(0) / np.float32(0)).astype(np.float32)

    from concourse.bass_utils import run_bass_kernel_spmd

    nc = _build_program(P, COLS)
    assert B <= N_CORES, "one batch per core"
    trace = os.environ.get("KERNEL_TRACE", "") not in ("", "0")
    if trace:
        _ensure_ntff_hook()
    res = run_bass_kernel_spmd(
        nc,
        in_maps,
        core_ids=list(range(B)),
        trace=trace,
    )
    LAST_RESULTS = res

    total = 0.0
    for b in range(B):
        out = res.results[b]["out"]
        # partials hold (upper + 0.5*diag); x2 recovers the full sum
        total += 2.0 * float(np.sum(out, dtype=np.float64))

    return np.asarray(
        np.float32(total) / np.float32(total_count), dtype=np.float32
    )


# revision 15
# speedup vs baseline: 1.4651x; 1.3310x over previous
"""Trainium2 Bass kernel for ComputeVecSimilarityLoss.

Reference semantics (B batches, N points, D=2):
    sm      = where(cos < th, 0, cos)                      [B,N,N]
    v[i,j]  = (gt[i] - gt[j]) * sm[i,j]  -> [B, M=N*N, D]
    dot     = v @ v^T per batch                            [B,M,M]
    idx_num = count(dot != 0)
    vabs    = sqrt(sum(v*v + 1e-9, axis=D))
    result  = sum(|dot| / (vabs_m*vabs_n)) / idx_num

Restructuring used here (mathematically exact, fp-equal to ~1e-6):
  * u = v / vabs  (host, O(B*M)): |dot|/(vabs_m*vabs_n) == |u_m . u_n|.
  * zero vectors (v == 0) contribute exactly 0.0 to every sum term and
    have dot == 0 for every pair, so they are compacted away on the host.
    idx_num = sum_b (nnz_b)^2  (the residual pairs have |dot| >> 0, no
    exact-cancellation; verified against the reference count).
  * batch b -> NeuronCore b (pure data parallel, B == 8 cores).
    Each core computes S_b = sum |u_m . u_n| over its compacted M_b x M_b
    block plus its nonzero count; host does the final scalar division.

Device kernel per core: PE emits K=2 matmuls (u^T u outer products) into
PSUM 256-column chunks; ScalarE (Abs activation with accum_out) and
VectorE (tensor_reduce with apply_absolute_value) split the |.|+sum work
over PSUM supertiles; per-partition partials are reduced on-chip to a
[128, 2] output (abs-dot partial sums, nz partial sums).
"""

import os

import numpy as np

EPS = np.float32(1e-9)
CHUNK = 256          # matmul free-dim per PSUM chunk (half bank of fp32)
GROUP_CHUNKS = 4     # chunks per PSUM supertile (2 banks = 1024 fp32)
N_CORES = 8
COLGROUPS = int(os.environ.get("KERNEL_COLGROUPS", "4"))

# Stash of the most recent BassKernelResults (for test harness profiling).
LAST_RESULTS = None

_PROGRAM_CACHE = {}


def _plan_groups(n_tiles, cols):
    """Plan triangular work and consumer assignment.

    The M x M |dot| matrix is symmetric.  Row tiles are processed in
    pairs p = {2p, 2p+1}; chunk index c (CHUNK=256 columns = 2 row
    tiles wide; the last chunk is narrowed to the real column count):
      * c == p: the 2x2 "superdiagonal" block, computed once, weight 1
        (handled on ScalarE with scale=0.5 so everything sums uniformly
        and the host multiplies the grand total by 2).
      * c > p: strictly above the superdiagonal, weight 2 (scale 1.0).
    Blocks below the superdiagonal are never computed.

    Groups of up to GROUP_CHUNKS equal-width chunks share one PSUM
    supertile and one consumer instruction.  Returns (plan, npart)
    where plan entries are (items, width, engine, col0, scale) and
    items are (row_tile, col_offset) pairs.
    """
    n_pairs = n_tiles // 2
    n_chunks = n_pairs  # 256-wide grid cells covering [0, 128*n_tiles)

    def chunk_w(c):
        return min(CHUNK, cols - c * CHUNK)

    d_work = [(2 * p + d, p) for p in range(n_pairs) for d in (0, 1)]
    u_work = [
        (2 * p + d, c)
        for p in range(n_pairs)
        for c in range(p + 1, n_chunks)
        for d in (0, 1)
    ]

    def cost_act(w):
        return (172.0 + w) / 1.2 + 283.0

    def cost_dve(w):
        # small fudge: measured DVE reduce runs ~3% over the model
        return 1.03 * (120.0 + w) / 0.96

    def group_by_width(work):
        """Split work into (items, width) groups of equal-width chunks."""
        by_w = {}
        for t, c in work:
            by_w.setdefault(chunk_w(c), []).append((t, c * CHUNK))
        out = []
        for w, items in by_w.items():
            for i in range(0, len(items), GROUP_CHUNKS):
                out.append((items[i : i + GROUP_CHUNKS], w))
        return out

    # superdiagonal groups must go to ScalarE (scale=0.5); balance the
    # upper groups across both engines by estimated cost.
    act_sts = []
    dve_sts = []
    act_ns = 0.0
    dve_ns = 0.0
    for items, w in group_by_width(d_work):
        act_sts.append((items, w, "act", 0.5))
        act_ns += cost_act(w * len(items))
    for items, w in group_by_width(u_work):
        if act_ns + cost_act(w * len(items)) <= dve_ns + cost_dve(w * len(items)):
            act_sts.append((items, w, "act", 1.0))
            act_ns += cost_act(w * len(items))
        else:
            dve_sts.append((items, w, "dve", 1.0))
            dve_ns += cost_dve(w * len(items))

    # big groups first, small last (shortens the final-consumer tail);
    # interleave the two queues so neither engine idles at the start
    # (emission order == PSUM fill order == consume order).
    act_sts.sort(key=lambda s: -len(s[0]) * s[1])
    dve_sts.sort(key=lambda s: -len(s[0]) * s[1])
    merged = []
    ia = idv = 0
    take_act = True
    while ia < len(act_sts) or idv < len(dve_sts):
        if take_act and ia < len(act_sts):
            merged.append(act_sts[ia])
            ia += 1
        elif idv < len(dve_sts):
            merged.append(dve_sts[idv])
            idv += 1
        else:
            merged.append(act_sts[ia])
            ia += 1
        take_act = not take_act

    plan = []
    col = 0
    for items, w, engine, scale in merged:
        plan.append((items, w, engine, col, scale))
        col += 1 if engine == "act" else len(items)
    return plan, col


def _build_program(P, COLS):
    """Build (and cache) the Bass program for padded size P x COLS."""
    key = (P, COLS)
    if key in _PROGRAM_CACHE:
        return _PROGRAM_CACHE[key]

    import concourse.bass as bass
    import concourse.mybir as mybir
    import concourse.tile as tile
    from concourse import bacc
    from concourse.bass import ts

    f32 = mybir.dt.float32
    f16 = mybir.dt.float16
    n_tiles = P // 128
    plan, npart = _plan_groups(n_tiles, COLS)

    nc = bacc.Bacc(
        "TRN2",
        target_bir_lowering=False,
        debug=False,
        enable_asserts=False,
        num_devices=N_CORES,
    )
    u_dram = nc.dram_tensor("u", [2, COLS], f16, kind="ExternalInput")
    nz_dram = nc.dram_tensor("nz", [128, n_tiles], f32, kind="ExternalInput")
    out_dram = nc.dram_tensor("out", [128, npart + 1], f32, kind="ExternalOutput")

    with tile.TileContext(nc) as tc:
        with (
            tc.tile_pool(name="const", bufs=1) as const_pool,
            tc.tile_pool(name="psum", bufs=4, space="PSUM") as psum_pool,
        ):
            # weights are sliced on the 128-row grid up to P, so pad the
            # SBUF copy of u to P columns and zero the tail beyond COLS
            u = const_pool.tile([2, P], f16)
            if COLS < P:
                nc.gpsimd.memset(u[:, COLS:P], 0.0)
            # split so the first matmuls can start before the whole of u lands
            split = min(512, COLS)
            nc.sync.dma_start(u[:, 0:split], u_dram.ap()[:, 0:split])
            if split < COLS:
                nc.sync.dma_start(u[:, split:COLS], u_dram.ap()[:, split:COLS])
            nzt = const_pool.tile([128, n_tiles], f32)
            nc.gpsimd.dma_start(nzt[:], nz_dram.ap())
            # partials columns [0:npart] = abs-dot sums; column npart = nz sum
            partials = const_pool.tile([128, npart + 1], f32)

            # nz reduction is independent of the main pipeline; do it early.
            nc.vector.reduce_sum(
                partials[:, npart : npart + 1], nzt[:], axis=mybir.AxisListType.X
            )

            for items, w, engine, col0, scale in plan:
                ps = psum_pool.tile([128, GROUP_CHUNKS * CHUNK], f32, tag="ps")
                for j, (t, off) in enumerate(items):
                    if COLGROUPS == 1:
                        nc.tensor.matmul(
                            ps[:, j * CHUNK : j * CHUNK + w],
                            u[:, ts(t, 128)],
                            u[:, off : off + w],
                        )
                    else:
                        # split the 128 output rows into col-group strips
                        # that execute concurrently on the PE array
                        m = 128 // COLGROUPS
                        for g in range(COLGROUPS):
                            nc.tensor.matmul(
                                ps[m * g : m * (g + 1), j * CHUNK : j * CHUNK + w],
                                u[:, t * 128 + m * g : t * 128 + m * (g + 1)],
                                u[:, off : off + w],
                                tile_position=(0, m * g),
                            )
                # chunks sit at CHUNK-strided offsets; view their first
                # w columns as a [128, n_chunks, w] strided AP
                ps3d = ps[:].rearrange("p (c n) -> p c n", n=CHUNK)[
                    :, 0 : len(items), 0:w
                ]
                if engine == "act":
                    nc.scalar.activation(
                        ps3d,
                        ps3d,
                        mybir.ActivationFunctionType.Abs,
                        scale=scale,
                        accum_out=partials[:, col0 : col0 + 1],
                    )
                else:
                    assert scale == 1.0
                    nc.vector.tensor_reduce(
                        partials[:, col0 : col0 + len(items)],
                        ps3d,
                        axis=mybir.AxisListType.X,
                        op=mybir.AluOpType.add,
                        apply_absolute_value=True,
                    )

            # ship raw per-partition partials; the host does the tiny final sum
            nc.sync.dma_start(out_dram.ap(), partials[:])

    nc.compile()
    _PROGRAM_CACHE[key] = nc
    return nc


def _preprocess(gt_points, cos_similarity, threshold):
    """Host O(B*N^2) prep: u vectors, compaction, padding.

    Returns (in_maps, nnz, P, COLS)."""
    gt = np.asarray(gt_points, dtype=np.float32)
    cos = np.asarray(cos_similarity, dtype=np.float32)
    th = np.asarray(threshold, dtype=np.float32).reshape(-1)[0]
    B, N, D = gt.shape
    M = N * N

    sm = np.where(cos < th, np.float32(0), cos)
    v = ((gt[:, :, None, :] - gt[:, None, :, :]) * sm[..., None]).reshape(B, M, D)
    v = v.astype(np.float32)
    # per-element eps, summed like the reference: (vx^2+eps) + (vy^2+eps)
    r2 = (v[..., 0] * v[..., 0] + EPS) + (v[..., 1] * v[..., 1] + EPS)
    vabs = np.sqrt(r2, dtype=np.float32)
    u = (v / vabs[..., None]).astype(np.float32)
    nz = np.any(v != 0, axis=-1)  # [B, M]
    nnz = nz.sum(axis=1).astype(np.int64)

    # COLS = real compacted width (rounded to 2 for fp16 alignment);
    # P = row padding to a multiple of 256 so the pair scheme has an
    # even tile count.  Columns beyond COLS are never touched.
    COLS = int(-(-max(2, int(nnz.max())) // 2) * 2)
    P = int(-(-COLS // CHUNK) * CHUNK)
    n_tiles = P // 128

    in_maps = []
    for b in range(B):
        ub = u[b][nz[b]]  # [nnz_b, 2] compacted
        ucols = np.zeros((2, COLS), dtype=np.float16)
        ucols[:, : ub.shape[0]] = ub.T.astype(np.float16)
        nzv = np.zeros(n_tiles * 128, dtype=np.float32)
        nzv[: ub.shape[0]] = 1.0
        nzdev = np.ascontiguousarray(nzv.reshape(n_tiles, 128).T)
        in_maps.append({"u": ucols, "nz": nzdev})
    return in_maps, nnz, P, COLS


def _ensure_ntff_hook():
    """Shim antenv.axon_hooks if the image lacks it (profiling only).

    Replicates trn_agent_boot._ntff_profile_via_ctypes against the baked
    libaxon_pjrt.so so run_bass_kernel_spmd(trace=True) can capture NTFFs.
    """
    try:
        from antenv.axon_hooks import get_axon_ntff_profile_hook  # noqa: F401

        return
    except ImportError:
        pass

    import contextlib
    import ctypes
    import sys
    import types

    import antenv

    mod = types.ModuleType("antenv.axon_hooks")
    _state = {"hook": None}

    def set_axon_ntff_profile_hook(h):
        _state["hook"] = h

    def get_axon_ntff_profile_hook():
        return _state["hook"]

    mod.set_axon_ntff_profile_hook = set_axon_ntff_profile_hook
    mod.get_axon_ntff_profile_hook = get_axon_ntff_profile_hook
    sys.modules["antenv.axon_hooks"] = mod
    antenv.axon_hooks = mod

    so_path = "/opt/axon/libaxon_pjrt.so"
    if not os.path.exists(so_path):
        return
    lib = ctypes.CDLL(so_path)
    if not hasattr(lib, "axon_start_nrt_profile"):
        return
    lib.axon_start_nrt_profile.argtypes = [
        ctypes.POINTER(ctypes.c_int64),
        ctypes.c_size_t,
    ]
    lib.axon_start_nrt_profile.restype = ctypes.c_int64
    lib.axon_stop_nrt_profile.argtypes = [ctypes.c_char_p]
    lib.axon_stop_nrt_profile.restype = ctypes.c_int64

    @contextlib.contextmanager
    def _hook(output_dir, device_ids):
        import jax

        jax.devices()
        if device_ids:
            ids = (ctypes.c_int64 * len(device_ids))(*device_ids)
            rc = lib.axon_start_nrt_profile(ids, len(device_ids))
        else:
            rc = lib.axon_start_nrt_profile(None, 0)
        if rc != 0:
            raise RuntimeError(f"axon_start_nrt_profile rc={rc}")
        try:
            yield
        finally:
            n = lib.axon_stop_nrt_profile(str(output_dir).encode())
            if n < 0:
                raise RuntimeError(f"axon_stop_nrt_profile rc={n}")
            print(f"profile: {n} file(s) written to {output_dir}")

    set_axon_ntff_profile_hook(_hook)


def kernel(gt_points, cos_similarity, threshold):
    global LAST_RESULTS
    in_maps, nnz, P, COLS = _preprocess(gt_points, cos_similarity, threshold)
    B = len(in_maps)

    total_count = int((nnz.astype(np.int64) ** 2).sum())
    if total_count == 0:
        # dot is identically zero: reference computes 0/0 in fp32.
        with np.errstate(invalid="ignore", divide="ignore"):
            return (np.float32(0) / np.float32(0)).astype(np.float32)

    from concourse.bass_utils import run_bass_kernel_spmd

    nc = _build_program(P, COLS)
    assert B <= N_CORES, "one batch per core"
    trace = os.environ.get("KERNEL_TRACE", "") not in ("", "0")
    if trace:
        _ensure_ntff_hook()
    res = run_bass_kernel_spmd(
        nc,
        in_maps,
        core_ids=list(range(B)),
        trace=trace,
    )
    LAST_RESULTS = res

    total = 0.0
    dev_count = 0
    for b in range(B):
        out = res.results[b]["out"]
        # partials hold (upper + 0.5*superdiag); x2 recovers the full sum
        total += 2.0 * float(np.sum(out[:, :-1], dtype=np.float64))
        nzs = int(round(float(np.sum(out[:, -1], dtype=np.float64))))
        dev_count += nzs * nzs
    assert dev_count == total_count, (dev_count, total_count)

    return np.asarray(
        np.float32(total) / np.float32(total_count), dtype=np.float32
    )

